# revision 44
# baseline (speedup 1.0000x reference)
"""Longformer attention Trainium2 kernel (8 NeuronCores, SPMD).

Sharding: data-parallel over batch (cores 0-3 -> batch 0, 4-7 -> batch 1),
head-parallel within a batch group (4 heads = 256 channels per core).

Q/K/V projections run as fp8 DoubleRow matmuls with an exact two-term
split: host ships x and 32*W as (hi, lo) fp8 pairs; DoubleRow computes
w0*m0 + w1*m1 per pass, so (Whi,Whi)x(xhi,xlo) + (Wlo,Wlo)x(xhi,xlo)
reconstructs the full product at 2x plain-DR cost with ~0.15% error.
The attention interior (scores, exp, PV, normalize, out-proj) is bf16.
Band masks are additive (-30) and applied on the tensor engine via
identity matmuls into the score PSUM.  Half the exps run on DVE via a
Schraudolph int16-bitcast approximation.  Softmax denominators come from
a ones-column in V; normalization happens on GPSIMD (partition_broadcast
+ multiply).  The epilogue (normalize/out-proj/store) of each pair is
software-pipelined three pairs deep so it never blocks later pairs in
the engines' in-order queues.  Host sums the 4 bf16 partials per batch
and adds the bias terms (bo and the fold of bv through Wo).  The K bias
is dropped on-device: it shifts every score in a softmax row by the same
constant, so softmax is invariant.
"""

import numpy as np
import ml_dtypes

import concourse.bacc as bacc
import concourse.mybir as mybir
from concourse.tile import TileContext
from concourse.bass_utils import run_bass_kernel_spmd

S = 2048          # sequence length
D = 1024          # model dim
NH = 16           # total heads
DH = 64           # head dim
HPC = 4           # heads per core
CPB = 4           # cores per batch
WIN = 256         # attention window (2 blocks of 128)
NB = S // 128     # 16 query/key blocks
WSC = 32.0        # weight pre-scale so fp8 split residuals stay normal
BF16 = mybir.dt.bfloat16
F32 = mybir.dt.float32
F8 = mybir.dt.float8e4
I16 = mybir.dt.int16
DR = mybir.MatmulPerfMode.DoubleRow
SCH_A = 128.0 / np.log(2.0)      # Schraudolph exp: bf16 = bitcast(i16(A*x + B))
SCH_B = 127.0 * 128.0 - 6.0

_CACHE = {}


def _band(qb):
    return list(range(max(0, qb - 2), min(NB - 1, qb + 2) + 1))


def _mask_id(qb, kb):
    # 0:M1 lower edge, 1:M1g (+global key row), 2:M2 upper edge, 3:M2g (+global query col)
    if kb == qb - 2:
        return 1 if kb == 0 else 0
    if kb == qb + 2:
        return 3 if qb == 0 else 2
    return None


def build_masks():
    """Additive masks (applied to scores in PSUM via identity matmul) plus
    the identity matrix itself in slot 4."""
    ki = np.arange(128)[:, None]
    qi = np.arange(128)[None, :]
    m1 = (qi <= ki).astype(np.float32)          # kb == qb-2 : valid iff qi <= ki
    m2 = (ki <= qi).astype(np.float32)          # kb == qb+2 : valid iff ki <= qi
    m1g = m1.copy(); m1g[0, :] = 1.0            # global key k=0 row
    m2g = m2.copy(); m2g[:, 0] = 1.0            # global query q=0 col
    mm = np.stack([m1, m1g, m2, m2g])
    add = -30.0 * (1.0 - mm)
    return np.concatenate([add, np.eye(128, dtype=np.float32)[None]],
                          axis=0).astype(ml_dtypes.bfloat16)


def build_program():
    nc = bacc.Bacc("TRN2", target_bir_lowering=False, debug=False, num_devices=8)

    xsd = nc.dram_tensor("xs", [128, 8, 2, S], F8, kind="ExternalInput").ap()
    whd = nc.dram_tensor("whi", [3, 128, 8, 2, 256], F8, kind="ExternalInput").ap()
    wld = nc.dram_tensor("wlo", [3, 128, 8, 2, 256], F8, kind="ExternalInput").ap()
    wod = nc.dram_tensor("wo", [128, 2, D], BF16, kind="ExternalInput").ap()
    bqd = nc.dram_tensor("bq", [2, 128, 1], F32, kind="ExternalInput").ap()
    maskd = nc.dram_tensor("masks", [5, 128, 128], BF16, kind="ExternalInput").ap()
    y = nc.dram_tensor("y", [S, D], BF16, kind="ExternalOutput").ap()

    with TileContext(nc) as tc:
        import contextlib
        with contextlib.ExitStack() as ctx, \
                nc.allow_low_precision(reason="fp8/bf16 attention interior by design"):
            sbw = ctx.enter_context(tc.tile_pool(name="sbw", bufs=1))
            sbx = ctx.enter_context(tc.tile_pool(name="sbx", bufs=1))
            sbqk = ctx.enter_context(tc.tile_pool(name="sbqk", bufs=1))
            sbes = ctx.enter_context(tc.tile_pool(name="sbes", bufs=8))
            sbsm = ctx.enter_context(tc.tile_pool(name="sbsm", bufs=4))
            sbbc = ctx.enter_context(tc.tile_pool(name="sbbc", bufs=9))
            sbrc = ctx.enter_context(tc.tile_pool(name="sbrc", bufs=4))
            psA = ctx.enter_context(tc.tile_pool(name="psA", bufs=2, space="PSUM"))
            psS = ctx.enter_context(tc.tile_pool(name="psS", bufs=2, space="PSUM"))
            psPV = ctx.enter_context(tc.tile_pool(name="psPV", bufs=2, space="PSUM"))

            # ---- load inputs ----
            wqh = sbw.tile([128, 8, 2, 256], F8, tag="wqh")
            nc.sync.dma_start(out=wqh[:], in_=whd[0, :, :, :, :])
            xs = sbx.tile([128, 8, 2, S], F8, tag="xs")
            nc.sync.dma_start(out=xs[:, :, :, 0:512], in_=xsd[:, :, :, 0:512])
            wkh = sbw.tile([128, 8, 2, 256], F8, tag="wkh")
            nc.sync.dma_start(out=wkh[:], in_=whd[1, :, :, :, :])
            wkl = sbw.tile([128, 8, 2, 256], F8, tag="wkl")
            nc.gpsimd.dma_start(out=wkl[:], in_=wld[1, :, :, :, :])
            nc.gpsimd.dma_start(out=xs[:, :, :, 512:1024], in_=xsd[:, :, :, 512:1024])
            bqt = []
            for cc in range(2):
                tq = sbw.tile([128, 1], F32, tag=f"bq{cc}")
                nc.sync.dma_start(out=tq[:], in_=bqd[cc, :, :])
                bqt.append(tq)
            mt = []
            for i in range(5):
                t = sbw.tile([128, 128], BF16, tag=f"mask{i}")
                nc.sync.dma_start(out=t[:], in_=maskd[i, :, :])
                mt.append(t)
            ident = mt[4]
            wvh = sbw.tile([128, 8, 2, 256], F8, tag="wvh")
            nc.sync.dma_start(out=wvh[:], in_=whd[2, :, :, :, :])
            wvl = sbw.tile([128, 8, 2, 256], F8, tag="wvl")
            nc.gpsimd.dma_start(out=wvl[:], in_=wld[2, :, :, :, :])
            nc.sync.dma_start(out=xs[:, :, :, 1024:1536], in_=xsd[:, :, :, 1024:1536])
            nc.gpsimd.dma_start(out=xs[:, :, :, 1536:2048], in_=xsd[:, :, :, 1536:2048])
            wot = sbw.tile([128, 2, D], BF16, tag="wo")
            nc.sync.dma_start(out=wot[:], in_=wod[:, :, :])

            # ---- persistent intermediates ----
            QT = [sbqk.tile([128, S], BF16, tag=f"QT{c}", name=f"QT{c}") for c in range(2)]
            KT = [sbqk.tile([128, S], BF16, tag=f"KT{c}", name=f"KT{c}") for c in range(2)]
            Vo = sbqk.tile([128, NB, HPC * 65], BF16, tag="Vo", name="Vo")
            AOb = sbqk.tile([128, 2, S], BF16, tag="AOb", name="AOb")

            # ---- projections: exact split fp8 DoubleRow, span-by-span ----
            def emit_qkt_span(ts):
                sp = slice(ts * 512, (ts + 1) * 512)
                for cc in range(2):
                    csl = slice(cc * 128, (cc + 1) * 128)
                    pq = psA.tile([128, 512], F32, tag="psA", name="pq")
                    for g in range(8):
                        nc.tensor.matmul(pq[:], wqh[:, g, :, csl],
                                         xs[:, g, :, sp],
                                         start=(g == 0), stop=(g == 7),
                                         perf_mode=DR)
                    # Q' = (x Wq + bq) / 8 : W pre-scaled by 32, bias by 1/8 on host
                    nc.vector.tensor_scalar(QT[cc][:, sp], pq[:], 0.125 / WSC,
                                            bqt[cc][:],
                                            mybir.AluOpType.mult, mybir.AluOpType.add)
                    pk = psA.tile([128, 512], F32, tag="psA", name="pk")
                    for i, wt in enumerate((wkh, wkl)):
                        for g in range(8):
                            nc.tensor.matmul(pk[:], wt[:, g, :, csl],
                                             xs[:, g, :, sp],
                                             start=(i == 0 and g == 0),
                                             stop=(i == 1 and g == 7), perf_mode=DR)
                    nc.scalar.activation(KT[cc][:, sp], pk[:],
                                         mybir.ActivationFunctionType.Copy,
                                         scale=1.0 / WSC)

            def emit_v(tb):
                tsl = slice(tb * 128, (tb + 1) * 128)
                pv = psA.tile([128, 256], F32, tag="psA", name="pv")
                for i, wt in enumerate((wvh, wvl)):
                    for g in range(8):
                        nc.tensor.matmul(pv[:], xs[:, g, :, tsl],
                                         wt[:, g, :, :],
                                         start=(i == 0 and g == 0),
                                         stop=(i == 1 and g == 7), perf_mode=DR)
                # scatter heads into [h*65 : h*65+64]; col h*65+64 gets ones
                vtb = Vo[:, tb, 0:260].rearrange("p (h c) -> p h c", h=4)
                inap = pv[:].rearrange("p (h c) -> p h c", h=4)
                nc.scalar.activation(vtb[:, :, 0:64], inap,
                                     mybir.ActivationFunctionType.Copy,
                                     scale=1.0 / WSC)
                nc.vector.memset(vtb[:, :, 64:65], 1.0)

            # ---- global key (k=0) score rows, batched 4 qb per exp ----
            # esgt[h][g] covers qb 4g..4g+3 as [1, 512]; only slices for qb>=3 used
            esgt = [[None] * 4 for _ in range(HPC)]
            def emit_esg(g):
                for h in range(HPC):
                    hp, r0 = h // 2, (h % 2) * 64
                    psg = psA.tile([128, 512], F32, tag="psA", name="psg")
                    for j in range(4):
                        qb = 4 * g + j
                        if qb < 3:
                            continue
                        nc.tensor.matmul(psg[0:1, j * 128:(j + 1) * 128],
                                         KT[hp][r0:r0 + 64, 0:1],
                                         QT[hp][r0:r0 + 64, qb * 128:(qb + 1) * 128],
                                         start=True, stop=True)
                    eg = sbsm.tile([1, 512], BF16, tag=f"esg{h}_{g}", name="eg")
                    lo = 3 if g == 0 else 0
                    nc.scalar.activation(eg[0:1, lo * 128:512], psg[0:1, lo * 128:512],
                                         mybir.ActivationFunctionType.Exp)
                    esgt[h][g] = eg

            emit_qkt_span(0)
            emit_esg(0)
            emit_qkt_span(1)
            emit_esg(1)

            # ---- banded attention; V tiles emitted just-in-time; pair 0
            # ---- (which needs all V for the global row) runs after pair 4
            pair_order = [1, 2, 3, 4, 0, 5, 6, 7]
            v_before = {1: range(0, 6), 2: range(6, 8), 3: range(8, 10),
                        4: range(10, 12), 0: range(12, 16)}

            # epilogue is software-pipelined three pairs deep so pair p's
            # normalize/out-proj/ys never block later pairs' scores/exp in
            # the engines' in-order queues
            def emit_epilogue(qb0, rec4, ao_tmps):
                psp = slice(qb0 * 128, (qb0 + 2) * 128)
                pbS = sbbc.tile([64, 1024], BF16, tag="pbS", name="pbS")
                nc.gpsimd.partition_broadcast(pbS[:], rec4[0:1, :])
                for h in range(HPC):
                    r0, cc = (h % 2) * 64, h // 2
                    nc.gpsimd.tensor_mul(AOb[r0:r0 + 64, cc, psp], ao_tmps[h][:],
                                         pbS[:, h * 256:(h + 1) * 256])
                for qb2 in (qb0, qb0 + 1):
                    q2 = slice(qb2 * 128, (qb2 + 1) * 128)
                    for eh in range(2):
                        po = psA.tile([128, 512], F32, tag="psA", name="po")
                        for cc in range(2):
                            nc.tensor.matmul(po[:], AOb[:, cc, q2],
                                             wot[:, cc, eh * 512:(eh + 1) * 512],
                                             start=(cc == 0), stop=(cc == 1))
                        ys = sbbc.tile([128, 512], BF16, tag="ystage", name="ys")
                        nc.scalar.activation(ys[:], po[:],
                                             mybir.ActivationFunctionType.Copy)
                        nc.sync.dma_start(out=y[q2, eh * 512:(eh + 1) * 512],
                                          in_=ys[:])

            pending = []
            for pair in pair_order:
                if pair == 2:
                    emit_qkt_span(2)
                    emit_esg(2)
                    emit_qkt_span(3)
                    emit_esg(3)
                for tb in v_before.get(pair, ()):
                    emit_v(tb)
                qb0 = pair * 2
                rec4 = sbrc.tile([1, 1024], BF16, tag="rec4", name="rec4")
                ao_tmps = {}
                for hp in range(2):
                    # both heads of the pair together: adjacent S^T matmuls hit
                    # different PE row-groups (partitions 0-63 vs 64-127) and
                    # overlap in the array
                    ppvs, jobs = {}, {0: [], 1: []}
                    for h2 in range(2):
                        ppvs[h2] = psPV.tile([65, 256], F32, tag="ppv", name="ppv")
                    for sub in range(2):
                        qb = qb0 + sub
                        qs = slice(qb * 128, (qb + 1) * 128)
                        kbs = _band(qb)
                        w = len(kbs) * 128
                        pss, ess = {}, {}
                        for h2 in range(2):
                            pss[h2] = psS.tile([128, 640], F32, tag="psS", name="ps")
                        for i, kb in enumerate(kbs):
                            for h2 in range(2):
                                r0 = h2 * 64
                                sl = slice(i * 128, (i + 1) * 128)
                                nc.tensor.matmul(pss[h2][:, sl],
                                                 KT[hp][r0:r0 + 64, kb * 128:(kb + 1) * 128],
                                                 QT[hp][r0:r0 + 64, qs],
                                                 start=True, stop=True)
                        # exp: split between Act (bf16 out) and DVE (Schraudolph
                        # int16-bitcast bf16)
                        dve_exp = {(0, 1), (1, 1)}
                        for h2 in range(2):
                            if (sub, h2) in dve_exp:
                                esi = sbes.tile([128, 640], I16, tag="es", name="esi")
                                nc.vector.tensor_scalar(esi[:, 0:w], pss[h2][:, 0:w],
                                                        SCH_A, SCH_B,
                                                        mybir.AluOpType.mult,
                                                        mybir.AluOpType.add)
                                esb = esi[:].bitcast(BF16)
                                for i, kb in enumerate(kbs):
                                    mid = _mask_id(qb, kb)
                                    if mid is not None:
                                        sl = slice(i * 128, (i + 1) * 128)
                                        nc.vector.tensor_mul(esb[:, sl], esb[:, sl],
                                                             mt[mid][:])
                                ess[h2] = esb
                            else:
                                es = sbes.tile([128, 640], BF16, tag="es", name="es")
                                nc.scalar.activation(es[:, 0:w], pss[h2][:, 0:w],
                                                     mybir.ActivationFunctionType.Exp)
                                for i, kb in enumerate(kbs):
                                    mid = _mask_id(qb, kb)
                                    if mid is not None:
                                        sl = slice(i * 128, (i + 1) * 128)
                                        nc.vector.tensor_mul(es[:, sl], es[:, sl],
                                                             mt[mid][:])
                                ess[h2] = es[:]
                        for h2 in range(2):
                            h = hp * 2 + h2
                            r0 = h2 * 64
                            hs = slice(h * 65, h * 65 + 65)
                            ov = ppvs[h2][:, sub * 128:(sub + 1) * 128]
                            esap = ess[h2]
                            for i, kb in enumerate(kbs):
                                jobs[h2].append((Vo[:, kb, hs],
                                                 esap[:, i * 128:(i + 1) * 128],
                                                 ov, i == 0, sub))
                            if qb >= 3:  # global key k=0 column
                                eg = esgt[h][qb // 4]
                                co = (qb % 4) * 128
                                jobs[h2].append((Vo[0:1, 0, hs], eg[0:1, co:co + 128],
                                                 ov, False, sub))
                            if qb == 0:  # global query q=0 vs far keys
                                ps0 = psA.tile([128, 512], F32, tag="psA", name="ps0")
                                for i, kb in enumerate(range(3, NB)):
                                    nc.tensor.matmul(
                                        ps0[:, i:i + 1],
                                        KT[hp][r0:r0 + 64, kb * 128:(kb + 1) * 128],
                                        QT[hp][r0:r0 + 64, 0:1], start=True, stop=True)
                                es0 = sbsm.tile([128, 13], BF16, tag="es0", name="es0")
                                nc.scalar.activation(es0[:], ps0[:, 0:13],
                                                     mybir.ActivationFunctionType.Exp)
                                for i, kb in enumerate(range(3, NB)):
                                    jobs[h2].append((Vo[:, kb, hs], es0[:, i:i + 1],
                                                     ppvs[h2][:, 0:1], False, sub))
                    for h2 in range(2):
                        h = hp * 2 + h2
                        pv_jobs = jobs[h2]
                        last_of_sub = {s: max(i for i, j in enumerate(pv_jobs)
                                              if j[4] == s) for s in (0, 1)}
                        for i_mm, (lh, rh, ov, first, sub) in enumerate(pv_jobs):
                            nc.tensor.matmul(ov, lh, rh, start=first,
                                             stop=(i_mm == last_of_sub[sub]))
                        nc.vector.reciprocal(rec4[0:1, h * 256:(h + 1) * 256],
                                             ppvs[h2][64:65, :])
                        ao_tmp = sbbc.tile([64, 256], BF16, tag="aotmp", name="ao_tmp")
                        nc.vector.tensor_copy(ao_tmp[:], ppvs[h2][0:64, :])
                        ao_tmps[h] = ao_tmp

                if len(pending) == 3:
                    emit_epilogue(*pending.pop(0))
                pending.append((qb0, rec4, ao_tmps))
            for p in pending:
                emit_epilogue(*p)

    nc.compile()
    return nc


def kernel(x, Wq, bq, Wk, bk, Wv, bv, Wo, bo):
    x = np.asarray(x); Wq = np.asarray(Wq); bq = np.asarray(bq)
    Wk = np.asarray(Wk); bk = np.asarray(bk); Wv = np.asarray(Wv)
    bv = np.asarray(bv); Wo = np.asarray(Wo); bo = np.asarray(bo)
    if "nc" not in _CACHE:
        _CACHE["nc"] = build_program()
    nc = _CACHE["nc"]

    B = x.shape[0]
    masks = build_masks()
    f8 = ml_dtypes.float8_e4m3
    bf = ml_dtypes.bfloat16

    def split8(a):
        h = a.astype(f8)
        l = (a - h.astype(np.float32)).astype(f8)
        return h, l

    in_maps = []
    for c in range(8):
        b = c // CPB
        h0 = (c % CPB) * HPC * DH          # channel offset of this core's heads
        sl = slice(h0, h0 + HPC * DH)
        # xs[p, g, t, n] = (hi,lo of x)[b][n, 128g+p]
        xt = np.ascontiguousarray(x[b].T.reshape(8, 128, S).transpose(1, 0, 2))
        xh, xl = split8(xt.astype(np.float32))
        xsarr = np.stack([xh, xl], axis=2)                # [128, 8, 2, S]
        whi = np.zeros((3, 128, 8, 2, 256), dtype=f8)
        wlo = np.zeros((3, 128, 8, 2, 256), dtype=f8)
        for wi, W in enumerate((Wq, Wk, Wv)):
            Ws = (W[:, sl] * WSC).astype(np.float32).reshape(8, 128, 256)
            Ws = Ws.transpose(1, 0, 2)                    # [128, 8, 256]
            h_, l_ = split8(Ws)
            whi[wi] = np.repeat(h_[:, :, None, :], 2, axis=2)
            wlo[wi] = np.repeat(l_[:, :, None, :], 2, axis=2)
        wo8 = np.ascontiguousarray(
            Wo[sl, :].reshape(2, 128, D).transpose(1, 0, 2)).astype(bf)
        in_maps.append({
            "xs": xsarr,
            "whi": whi,
            "wlo": wlo,
            "wo": wo8,
            "bq": (bq[sl] * 0.125).reshape(2, 128, 1).astype(np.float32),
            "masks": masks,
        })
    res = run_bass_kernel_spmd(nc, in_maps, list(range(8)))
    out = np.zeros((B, S, D), dtype=np.float32)
    for c in range(8):
        out[c // CPB] += res.results[c]["y"].astype(np.float32)
    out += (bv @ Wo + bo)[None, None, :]
    return out


# revision 45
# speedup vs baseline: 1.0231x; 1.0231x over previous
"""Longformer attention Trainium2 kernel (8 NeuronCores, SPMD).

Sharding: data-parallel over batch (cores 0-3 -> batch 0, 4-7 -> batch 1),
head-parallel within a batch group (4 heads = 256 channels per core).

Q/K/V projections run as fp8 DoubleRow matmuls with an exact two-term
split: host ships x and 32*W as (hi, lo) fp8 pairs; DoubleRow computes
w0*m0 + w1*m1 per pass, so (Whi,Whi)x(xhi,xlo) + (Wlo,Wlo)x(xhi,xlo)
reconstructs the full product at 2x plain-DR cost with ~0.15% error.
The attention interior (scores, exp, PV, normalize, out-proj) is bf16.
Band masks are additive (-30) and applied on the tensor engine via
identity matmuls into the score PSUM.  Half the exps run on DVE via a
Schraudolph int16-bitcast approximation.  Softmax denominators come from
a ones-column in V; normalization happens on GPSIMD (partition_broadcast
+ multiply).  The epilogue (normalize/out-proj/store) of each pair is
software-pipelined three pairs deep so it never blocks later pairs in
the engines' in-order queues.  Host sums the 4 bf16 partials per batch
and adds the bias terms (bo and the fold of bv through Wo).  The K bias
is dropped on-device: it shifts every score in a softmax row by the same
constant, so softmax is invariant.
"""

import numpy as np
import ml_dtypes

import concourse.bacc as bacc
import concourse.mybir as mybir
from concourse.tile import TileContext
from concourse.bass_utils import run_bass_kernel_spmd

S = 2048          # sequence length
D = 1024          # model dim
NH = 16           # total heads
DH = 64           # head dim
HPC = 4           # heads per core
CPB = 4           # cores per batch
WIN = 256         # attention window (2 blocks of 128)
NB = S // 128     # 16 query/key blocks
WSC = 32.0        # weight pre-scale so fp8 split residuals stay normal
BF16 = mybir.dt.bfloat16
F32 = mybir.dt.float32
F8 = mybir.dt.float8e4
I16 = mybir.dt.int16
DR = mybir.MatmulPerfMode.DoubleRow
SCH_A = 128.0 / np.log(2.0)      # Schraudolph exp: bf16 = bitcast(i16(A*x + B))
SCH_B = 127.0 * 128.0 - 6.0

_CACHE = {}


def _band(qb):
    return list(range(max(0, qb - 2), min(NB - 1, qb + 2) + 1))


def _mask_id(qb, kb):
    # 0:M1 lower edge, 1:M1g (+global key row), 2:M2 upper edge, 3:M2g (+global query col)
    if kb == qb - 2:
        return 1 if kb == 0 else 0
    if kb == qb + 2:
        return 3 if qb == 0 else 2
    return None


def build_masks():
    """Additive masks (applied to scores in PSUM via identity matmul) plus
    the identity matrix itself in slot 4."""
    ki = np.arange(128)[:, None]
    qi = np.arange(128)[None, :]
    m1 = (qi <= ki).astype(np.float32)          # kb == qb-2 : valid iff qi <= ki
    m2 = (ki <= qi).astype(np.float32)          # kb == qb+2 : valid iff ki <= qi
    m1g = m1.copy(); m1g[0, :] = 1.0            # global key k=0 row
    m2g = m2.copy(); m2g[:, 0] = 1.0            # global query q=0 col
    mm = np.stack([m1, m1g, m2, m2g])
    add = -30.0 * (1.0 - mm)
    return np.concatenate([add, np.eye(128, dtype=np.float32)[None]],
                          axis=0).astype(ml_dtypes.bfloat16)


def build_program():
    nc = bacc.Bacc("TRN2", target_bir_lowering=False, debug=False, num_devices=8)

    xsd = nc.dram_tensor("xs", [128, 8, 2, S], F8, kind="ExternalInput").ap()
    whd = nc.dram_tensor("whi", [3, 128, 8, 2, 256], F8, kind="ExternalInput").ap()
    wld = nc.dram_tensor("wlo", [3, 128, 8, 2, 256], F8, kind="ExternalInput").ap()
    wod = nc.dram_tensor("wo", [128, 2, D], BF16, kind="ExternalInput").ap()
    bqd = nc.dram_tensor("bq", [2, 128, 1], F32, kind="ExternalInput").ap()
    maskd = nc.dram_tensor("masks", [5, 128, 128], BF16, kind="ExternalInput").ap()
    y = nc.dram_tensor("y", [S, D], BF16, kind="ExternalOutput").ap()

    with TileContext(nc) as tc:
        import contextlib
        with contextlib.ExitStack() as ctx, \
                nc.allow_low_precision(reason="fp8/bf16 attention interior by design"):
            sbw = ctx.enter_context(tc.tile_pool(name="sbw", bufs=1))
            sbx = ctx.enter_context(tc.tile_pool(name="sbx", bufs=1))
            sbqk = ctx.enter_context(tc.tile_pool(name="sbqk", bufs=1))
            sbes = ctx.enter_context(tc.tile_pool(name="sbes", bufs=8))
            sbsm = ctx.enter_context(tc.tile_pool(name="sbsm", bufs=4))
            sbbc = ctx.enter_context(tc.tile_pool(name="sbbc", bufs=9))
            sbrc = ctx.enter_context(tc.tile_pool(name="sbrc", bufs=4))
            psA = ctx.enter_context(tc.tile_pool(name="psA", bufs=2, space="PSUM"))
            psS = ctx.enter_context(tc.tile_pool(name="psS", bufs=2, space="PSUM"))
            psPV = ctx.enter_context(tc.tile_pool(name="psPV", bufs=2, space="PSUM"))

            # ---- load inputs ----
            wqh = sbw.tile([128, 8, 2, 256], F8, tag="wqh")
            nc.sync.dma_start(out=wqh[:], in_=whd[0, :, :, :, :])
            xs = sbx.tile([128, 8, 2, S], F8, tag="xs")
            nc.sync.dma_start(out=xs[:, :, :, 0:512], in_=xsd[:, :, :, 0:512])
            wkh = sbw.tile([128, 8, 2, 256], F8, tag="wkh")
            nc.sync.dma_start(out=wkh[:], in_=whd[1, :, :, :, :])
            wkl = sbw.tile([128, 8, 2, 256], F8, tag="wkl")
            nc.gpsimd.dma_start(out=wkl[:], in_=wld[1, :, :, :, :])
            nc.gpsimd.dma_start(out=xs[:, :, :, 512:1024], in_=xsd[:, :, :, 512:1024])
            bqt = []
            for cc in range(2):
                tq = sbw.tile([128, 1], F32, tag=f"bq{cc}")
                nc.sync.dma_start(out=tq[:], in_=bqd[cc, :, :])
                bqt.append(tq)
            mt = []
            for i in range(5):
                t = sbw.tile([128, 128], BF16, tag=f"mask{i}")
                nc.sync.dma_start(out=t[:], in_=maskd[i, :, :])
                mt.append(t)
            ident = mt[4]
            wvh = sbw.tile([128, 8, 2, 256], F8, tag="wvh")
            nc.sync.dma_start(out=wvh[:], in_=whd[2, :, :, :, :])
            wvl = sbw.tile([128, 8, 2, 256], F8, tag="wvl")
            nc.gpsimd.dma_start(out=wvl[:], in_=wld[2, :, :, :, :])
            nc.sync.dma_start(out=xs[:, :, :, 1024:1536], in_=xsd[:, :, :, 1024:1536])
            nc.gpsimd.dma_start(out=xs[:, :, :, 1536:2048], in_=xsd[:, :, :, 1536:2048])
            wot = sbw.tile([128, 2, D], BF16, tag="wo")
            nc.sync.dma_start(out=wot[:], in_=wod[:, :, :])

            # ---- persistent intermediates ----
            QT = [sbqk.tile([128, S], BF16, tag=f"QT{c}", name=f"QT{c}") for c in range(2)]
            KT = [sbqk.tile([128, S], BF16, tag=f"KT{c}", name=f"KT{c}") for c in range(2)]
            Vo = sbqk.tile([128, NB, HPC * 65], BF16, tag="Vo", name="Vo")
            AOb = sbqk.tile([128, 2, S], BF16, tag="AOb", name="AOb")

            # ---- projections: exact split fp8 DoubleRow, span-by-span ----
            def emit_qkt_span(ts):
                sp = slice(ts * 512, (ts + 1) * 512)
                for cc in range(2):
                    csl = slice(cc * 128, (cc + 1) * 128)
                    pq = psA.tile([128, 512], F32, tag="psA", name="pq")
                    for g in range(8):
                        nc.tensor.matmul(pq[:], wqh[:, g, :, csl],
                                         xs[:, g, :, sp],
                                         start=(g == 0), stop=(g == 7),
                                         perf_mode=DR)
                    # Q' = (x Wq + bq) / 8 : W pre-scaled by 32, bias by 1/8 on host
                    nc.vector.tensor_scalar(QT[cc][:, sp], pq[:], 0.125 / WSC,
                                            bqt[cc][:],
                                            mybir.AluOpType.mult, mybir.AluOpType.add)
                    pk = psA.tile([128, 512], F32, tag="psA", name="pk")
                    for i, wt in enumerate((wkh, wkl)):
                        for g in range(8):
                            nc.tensor.matmul(pk[:], wt[:, g, :, csl],
                                             xs[:, g, :, sp],
                                             start=(i == 0 and g == 0),
                                             stop=(i == 1 and g == 7), perf_mode=DR)
                    nc.scalar.activation(KT[cc][:, sp], pk[:],
                                         mybir.ActivationFunctionType.Copy,
                                         scale=1.0 / WSC)

            def emit_v(tb):
                tsl = slice(tb * 128, (tb + 1) * 128)
                pv = psA.tile([128, 256], F32, tag="psA", name="pv")
                for i, wt in enumerate((wvh, wvl)):
                    for g in range(8):
                        nc.tensor.matmul(pv[:], xs[:, g, :, tsl],
                                         wt[:, g, :, :],
                                         start=(i == 0 and g == 0),
                                         stop=(i == 1 and g == 7), perf_mode=DR)
                # scatter heads into [h*65 : h*65+64]; col h*65+64 gets ones
                vtb = Vo[:, tb, 0:260].rearrange("p (h c) -> p h c", h=4)
                inap = pv[:].rearrange("p (h c) -> p h c", h=4)
                nc.scalar.activation(vtb[:, :, 0:64], inap,
                                     mybir.ActivationFunctionType.Copy,
                                     scale=1.0 / WSC)
                nc.vector.memset(vtb[:, :, 64:65], 1.0)

            # ---- global key (k=0) score rows, batched 4 qb per exp ----
            # esgt[h][g] covers qb 4g..4g+3 as [1, 512]; only slices for qb>=3 used
            esgt = [[None] * 4 for _ in range(HPC)]
            def emit_esg(g):
                for h in range(HPC):
                    hp, r0 = h // 2, (h % 2) * 64
                    psg = psA.tile([128, 512], F32, tag="psA", name="psg")
                    for j in range(4):
                        qb = 4 * g + j
                        if qb < 3:
                            continue
                        nc.tensor.matmul(psg[0:1, j * 128:(j + 1) * 128],
                                         KT[hp][r0:r0 + 64, 0:1],
                                         QT[hp][r0:r0 + 64, qb * 128:(qb + 1) * 128],
                                         start=True, stop=True)
                    eg = sbsm.tile([1, 512], BF16, tag=f"esg{h}_{g}", name="eg")
                    lo = 3 if g == 0 else 0
                    nc.scalar.activation(eg[0:1, lo * 128:512], psg[0:1, lo * 128:512],
                                         mybir.ActivationFunctionType.Exp)
                    esgt[h][g] = eg

            emit_qkt_span(0)
            emit_esg(0)
            emit_qkt_span(1)
            emit_esg(1)

            # ---- banded attention; V tiles emitted just-in-time; pair 0
            # ---- (which needs all V for the global row) runs after pair 4
            pair_order = [1, 2, 3, 4, 0, 5, 6, 7]
            v_before = {1: range(0, 6), 2: range(6, 8), 3: range(8, 10),
                        4: range(10, 12), 0: range(12, 16)}

            # epilogue is software-pipelined three pairs deep so pair p's
            # normalize/out-proj/ys never block later pairs' scores/exp in
            # the engines' in-order queues
            def emit_epilogue(qb0, rec4, ao_tmps):
                psp = slice(qb0 * 128, (qb0 + 2) * 128)
                pbS = sbbc.tile([64, 1024], BF16, tag="pbS", name="pbS")
                nc.gpsimd.partition_broadcast(pbS[:], rec4[0:1, :])
                for h in range(HPC):
                    r0, cc = (h % 2) * 64, h // 2
                    nc.gpsimd.tensor_mul(AOb[r0:r0 + 64, cc, psp], ao_tmps[h][:],
                                         pbS[:, h * 256:(h + 1) * 256])
                for qb2 in (qb0, qb0 + 1):
                    q2 = slice(qb2 * 128, (qb2 + 1) * 128)
                    for eh in range(2):
                        po = psA.tile([128, 512], F32, tag="psA", name="po")
                        for cc in range(2):
                            nc.tensor.matmul(po[:], AOb[:, cc, q2],
                                             wot[:, cc, eh * 512:(eh + 1) * 512],
                                             start=(cc == 0), stop=(cc == 1))
                        ys = sbbc.tile([128, 512], BF16, tag="ystage", name="ys")
                        nc.scalar.activation(ys[:], po[:],
                                             mybir.ActivationFunctionType.Copy)
                        nc.sync.dma_start(out=y[q2, eh * 512:(eh + 1) * 512],
                                          in_=ys[:])

            pending = []
            for pair in pair_order:
                if pair == 2:
                    emit_qkt_span(2)
                    emit_esg(2)
                    emit_qkt_span(3)
                    emit_esg(3)
                for tb in v_before.get(pair, ()):
                    emit_v(tb)
                qb0 = pair * 2
                rec4 = sbrc.tile([1, 1024], BF16, tag="rec4", name="rec4")
                ao_tmps = {}
                for hp in range(2):
                    # both heads of the pair together: adjacent S^T matmuls hit
                    # different PE row-groups (partitions 0-63 vs 64-127) and
                    # overlap in the array
                    ppvs, jobs = {}, {0: [], 1: []}
                    for h2 in range(2):
                        ppvs[h2] = psPV.tile([65, 256], F32, tag="ppv", name="ppv")
                    for sub in range(2):
                        qb = qb0 + sub
                        qs = slice(qb * 128, (qb + 1) * 128)
                        kbs = _band(qb)
                        w = len(kbs) * 128
                        pss, ess = {}, {}
                        for h2 in range(2):
                            pss[h2] = psS.tile([128, 640], F32, tag="psS", name="ps")
                        for i, kb in enumerate(kbs):
                            mid = _mask_id(qb, kb)
                            for h2 in range(2):
                                r0 = h2 * 64
                                sl = slice(i * 128, (i + 1) * 128)
                                nc.tensor.matmul(pss[h2][:, sl],
                                                 KT[hp][r0:r0 + 64, kb * 128:(kb + 1) * 128],
                                                 QT[hp][r0:r0 + 64, qs],
                                                 start=True, stop=(mid is None))
                                if mid is not None:
                                    # additive mask: out += I.T @ mask = mask
                                    nc.tensor.matmul(pss[h2][:, sl], ident[:],
                                                     mt[mid][:], start=False, stop=True)
                        # exp: split between Act (bf16 out) and DVE (Schraudolph
                        # int16-bitcast bf16)
                        dve_exp = {(0, 1), (1, 1)}
                        for h2 in range(2):
                            if (sub, h2) in dve_exp:
                                esi = sbes.tile([128, 640], I16, tag="es", name="esi")
                                nc.vector.tensor_scalar(esi[:, 0:w], pss[h2][:, 0:w],
                                                        SCH_A, SCH_B,
                                                        mybir.AluOpType.mult,
                                                        mybir.AluOpType.add)
                                ess[h2] = esi[:].bitcast(BF16)
                            else:
                                es = sbes.tile([128, 640], BF16, tag="es", name="es")
                                nc.scalar.activation(es[:, 0:w], pss[h2][:, 0:w],
                                                     mybir.ActivationFunctionType.Exp)
                                ess[h2] = es[:]
                        for h2 in range(2):
                            h = hp * 2 + h2
                            r0 = h2 * 64
                            hs = slice(h * 65, h * 65 + 65)
                            ov = ppvs[h2][:, sub * 128:(sub + 1) * 128]
                            esap = ess[h2]
                            for i, kb in enumerate(kbs):
                                jobs[h2].append((Vo[:, kb, hs],
                                                 esap[:, i * 128:(i + 1) * 128],
                                                 ov, i == 0, sub))
                            if qb >= 3:  # global key k=0 column
                                eg = esgt[h][qb // 4]
                                co = (qb % 4) * 128
                                jobs[h2].append((Vo[0:1, 0, hs], eg[0:1, co:co + 128],
                                                 ov, False, sub))
                            if qb == 0:  # global query q=0 vs far keys
                                ps0 = psA.tile([128, 512], F32, tag="psA", name="ps0")
                                for i, kb in enumerate(range(3, NB)):
                                    nc.tensor.matmul(
                                        ps0[:, i:i + 1],
                                        KT[hp][r0:r0 + 64, kb * 128:(kb + 1) * 128],
                                        QT[hp][r0:r0 + 64, 0:1], start=True, stop=True)
                                es0 = sbsm.tile([128, 13], BF16, tag="es0", name="es0")
                                nc.scalar.activation(es0[:], ps0[:, 0:13],
                                                     mybir.ActivationFunctionType.Exp)
                                for i, kb in enumerate(range(3, NB)):
                                    jobs[h2].append((Vo[:, kb, hs], es0[:, i:i + 1],
                                                     ppvs[h2][:, 0:1], False, sub))
                    for h2 in range(2):
                        h = hp * 2 + h2
                        pv_jobs = jobs[h2]
                        last_of_sub = {s: max(i for i, j in enumerate(pv_jobs)
                                              if j[4] == s) for s in (0, 1)}
                        for i_mm, (lh, rh, ov, first, sub) in enumerate(pv_jobs):
                            nc.tensor.matmul(ov, lh, rh, start=first,
                                             stop=(i_mm == last_of_sub[sub]))
                        nc.vector.reciprocal(rec4[0:1, h * 256:(h + 1) * 256],
                                             ppvs[h2][64:65, :])
                        ao_tmp = sbbc.tile([64, 256], BF16, tag="aotmp", name="ao_tmp")
                        nc.vector.tensor_copy(ao_tmp[:], ppvs[h2][0:64, :])
                        ao_tmps[h] = ao_tmp

                if len(pending) == 3:
                    emit_epilogue(*pending.pop(0))
                pending.append((qb0, rec4, ao_tmps))
            for p in pending:
                emit_epilogue(*p)

    nc.compile()
    return nc


def kernel(x, Wq, bq, Wk, bk, Wv, bv, Wo, bo):
    x = np.asarray(x); Wq = np.asarray(Wq); bq = np.asarray(bq)
    Wk = np.asarray(Wk); bk = np.asarray(bk); Wv = np.asarray(Wv)
    bv = np.asarray(bv); Wo = np.asarray(Wo); bo = np.asarray(bo)
    if "nc" not in _CACHE:
        _CACHE["nc"] = build_program()
    nc = _CACHE["nc"]

    B = x.shape[0]
    masks = build_masks()
    f8 = ml_dtypes.float8_e4m3
    bf = ml_dtypes.bfloat16

    def split8(a):
        h = a.astype(f8)
        l = (a - h.astype(np.float32)).astype(f8)
        return h, l

    in_maps = []
    for c in range(8):
        b = c // CPB
        h0 = (c % CPB) * HPC * DH          # channel offset of this core's heads
        sl = slice(h0, h0 + HPC * DH)
        # xs[p, g, t, n] = (hi,lo of x)[b][n, 128g+p]
        xt = np.ascontiguousarray(x[b].T.reshape(8, 128, S).transpose(1, 0, 2))
        xh, xl = split8(xt.astype(np.float32))
        xsarr = np.stack([xh, xl], axis=2)                # [128, 8, 2, S]
        whi = np.zeros((3, 128, 8, 2, 256), dtype=f8)
        wlo = np.zeros((3, 128, 8, 2, 256), dtype=f8)
        for wi, W in enumerate((Wq, Wk, Wv)):
            Ws = (W[:, sl] * WSC).astype(np.float32).reshape(8, 128, 256)
            Ws = Ws.transpose(1, 0, 2)                    # [128, 8, 256]
            h_, l_ = split8(Ws)
            whi[wi] = np.repeat(h_[:, :, None, :], 2, axis=2)
            wlo[wi] = np.repeat(l_[:, :, None, :], 2, axis=2)
        wo8 = np.ascontiguousarray(
            Wo[sl, :].reshape(2, 128, D).transpose(1, 0, 2)).astype(bf)
        in_maps.append({
            "xs": xsarr,
            "whi": whi,
            "wlo": wlo,
            "wo": wo8,
            "bq": (bq[sl] * 0.125).reshape(2, 128, 1).astype(np.float32),
            "masks": masks,
        })
    res = run_bass_kernel_spmd(nc, in_maps, list(range(8)))
    out = np.zeros((B, S, D), dtype=np.float32)
    for c in range(8):
        out[c // CPB] += res.results[c]["y"].astype(np.float32)
    out += (bv @ Wo + bo)[None, None, :]
    return out
